# revision 11
# baseline (speedup 1.0000x reference)
"""DeepStitch Trainium2 Bass kernel (8-core, full-B replication), v2.

Core 2b+par owns image b and descriptor half par.  Per core:
  conv A on its own row-half, n-PARTITIONED into an SBUF-resident feature
  bank fanall [128, 64, 256] fp16 (partition = n%128, rank = n//128) with
  relu alternating ACT/Pool; resp row-sums on DVE (fp32, exact vs ref) ->
  packed-bits selection tree (unchanged) -> 128 descriptor indices ->
  SBUF-source dma_gather pulls descriptors c-partitioned [c, mb, k] ->
  conv B (full image, 16 groups of 1024 positions) fine-interleaved with
  16 scoring chunks on PE so the tensor engine never stalls; relu mb0/mb1
  split ACT/Pool, squares on DVE (fp16 2x); per-chunk top-1 via a single
  DVE Max, with the chunk psum spilled fp32 to DRAM in parallel; global
  winner = Max/MaxIndex over the 16 chunk maxima, then one dma_gather
  re-fetches each k's winning 1024-chunk and one MaxIndex recovers the
  exact argmin index -> displacements -> single pairwise AllGather ->
  row/col MLPs (host reads even cores' out).

Conv bias rides as a 28th im2col row; conv-A weights are x4 and conv-B x2
so gathered descriptors arrive pre-scaled (desc.fb - |fb|^2 = 4*(2ab-b^2),
argmin-equivalent).  Conv matmuls in float32r; scoring in fp16 (index-
exact vs the fp32 reference).  All constants ride in two host-packed blob
DMAs; im2col loads are single 28-descriptor DMAs from host-prepped
stride-2 tap planes.
"""

import sys

for _p in ("/opt/trn_rl_repo",):
    if _p not in sys.path:
        sys.path.insert(0, _p)

import numpy as np

import concourse.bacc as bacc
import concourse.bass as bass
import concourse.mybir as mybir
import concourse.tile as tile
import concourse.bass_utils as bass_utils
from concourse.bass import AP
from contextlib import ExitStack

F32 = mybir.dt.float32
F32R = mybir.dt.float32r
F16 = mybir.dt.float16
I16 = mybir.dt.int16
I32 = mybir.dt.int32
U32 = mybir.dt.uint32
AF = mybir.ActivationFunctionType
ALU = mybir.AluOpType

B = 4
NCORES = 8
CIN = 3
COUT = 256
H = W = 128
NH = 8192           # half-image positions (conv-A share)
NF = 16384          # full-image positions (conv-B)

_DYS = {0: [0, 2], 1: [1]}

# ---- const blob layout (f32 columns) ----
CB_W27A = 0          # [32, 256] f32r
CB_W27 = 256         # [32, 256] f32r
CB_ROWBL = 512       # [128, 1] i32
CB_COLB = 513        # [128, 1] i32
CB_ROW64 = 514       # [128, 1] i32
CB_KIOTA = 515       # [128, 1] i32 (= partition index k)
CB_RPAT = 516        # [128, 64] i32
CB_W1 = 580          # [128, 2, 2, 128] f32
CB_B1 = 1092         # [128, 2] f32
CB_W2 = 1094         # [128, 2] f32
CB_B2 = 1096         # [1, 2] f32 (partition 0)
CB_COLS = 1098


def _tap_order():
    taps = []
    for pr in (0, 1):
        for pc in (0, 1):
            for c in range(CIN):
                for dy in _DYS[pr]:
                    for dx in _DYS[pc]:
                        taps.append((c, dy, dx))
    assert len(taps) == 27
    return taps


TAPS = _tap_order()


def _prep_planes(x, r0, nrows):
    """[3,256,256] -> per-tap stride-2 planes [28, nrows*128]; row 27 = ones
    (bias row, pairs with the bias row of the weight matrix)."""
    xp = np.zeros((CIN, 259, 259), dtype=np.float32)
    xp[:, 1:257, 1:257] = x
    out = np.empty((28, nrows, 128), dtype=np.float32)
    for t, (c, dy, dx) in enumerate(TAPS):
        sub = xp[c, dy::2, dx::2]
        out[t] = sub[r0 : r0 + nrows, :128]
    out[27] = 1.0
    return out.reshape(28, nrows * 128)


def _prep_w27(Wconv, bconv, scale):
    w = np.zeros((32, COUT), dtype=np.float32)
    for i, (c, dy, dx) in enumerate(TAPS):
        w[i] = scale * Wconv[:, c, dy, dx]
    w[27] = scale * bconv
    return w


def build_kernel(dbg=False):
    nc = bacc.Bacc("TRN2", target_bir_lowering=False, debug=False,
                   num_devices=NCORES)

    blob32 = nc.dram_tensor("blob32", [128, CB_COLS], F32, kind="ExternalInput")
    blob16 = nc.dram_tensor("blob16", [128, 128], F16, kind="ExternalInput")
    wblob = nc.dram_tensor("wblob", [32, 512], F32R, kind="ExternalInput")
    xa = nc.dram_tensor("xa", [28, NH], F32R, kind="ExternalInput")
    xb = nc.dram_tensor("xb", [28, NF], F32R, kind="ExternalInput")
    out = nc.dram_tensor("out", [1, 2], F32, kind="ExternalOutput")
    scr = nc.dram_tensor("scr", [128], I16, kind="Internal")
    scr2 = nc.dram_tensor("scr2", [128], I16, kind="Internal")
    rvd = nc.dram_tensor("rvd", [2048], F32, kind="Internal")
    fa_t = nc.dram_tensor("fa_t", [NH, COUT], F16, kind="Internal")
    scd = nc.dram_tensor("scd", [2048, 1024], F32, kind="Internal")

    if dbg:
        na_dbg = nc.dram_tensor("na_dbg", [128, 1], I32, kind="ExternalOutput")
        desc_dbg = nc.dram_tensor("desc_dbg", [128, 2, 128], F16, kind="ExternalOutput")
        nb_dbg = nc.dram_tensor("nb_dbg", [128, 1], I32, kind="ExternalOutput")
        drow_dbg = nc.dram_tensor("drow_dbg", [128, 2], F32, kind="ExternalOutput")

    with tile.TileContext(nc) as tc, ExitStack() as ctx:
        const = ctx.enter_context(tc.tile_pool(name="const", bufs=1))
        small = ctx.enter_context(tc.tile_pool(name="small", bufs=1))
        feat = ctx.enter_context(tc.tile_pool(name="feat", bufs=1))
        ima_pool = ctx.enter_context(tc.tile_pool(name="ima", bufs=2))
        imb_pool = ctx.enter_context(tc.tile_pool(name="imb", bufs=4))
        fbt_pool = ctx.enter_context(tc.tile_pool(name="fbt", bufs=4))
        dram = ctx.enter_context(tc.tile_pool(name="dram", bufs=1, space="DRAM"))
        cq_pool = ctx.enter_context(tc.tile_pool(name="cq", bufs=2, space="PSUM"))
        cq2_pool = ctx.enter_context(tc.tile_pool(name="cq2", bufs=2, space="PSUM"))

        # ---- head: all input DMAs on the SP queue, deps-free, in use order ----
        cb32 = const.tile([128, CB_COLS], F32, tag="cb32")
        cb16 = const.tile([128, 128], F16, tag="cb16")
        cwb = const.tile([32, 512], F32R, tag="cwb")
        nc.sync.dma_start(cwb[:], wblob.ap())
        nc.sync.dma_start(cb32[:], blob32.ap())
        im_a = []
        for ha in range(2):
            t = ima_pool.tile([32, NH // 2], F32R, tag="ima")
            nc.sync.dma_start(
                t[0:28, :], AP(tensor=xa, offset=ha * (NH // 2),
                               ap=[[NH, 28], [1, NH // 2]]))
            im_a.append(t)
        nc.sync.dma_start(cb16[:], blob16.ap())
        im_b = []
        for hb in range(4):
            t = imb_pool.tile([32, 4096], F32R, tag="imb")
            nc.sync.dma_start(
                t[0:28, :], AP(tensor=xb, offset=hb * 4096,
                               ap=[[NF, 28], [1, 4096]]))
            im_b.append(t)

        # const views
        w27a_v = cwb[0:28, 0:256]
        w27_v = cwb[0:28, 256:512]
        rowbl_v = cb32[:, CB_ROWBL : CB_ROWBL + 1].bitcast(I32)
        colb_v = cb32[:, CB_COLB : CB_COLB + 1].bitcast(I32)
        row64_v = cb32[:, CB_ROW64 : CB_ROW64 + 1].bitcast(I32)
        kiota_v = cb32[:, CB_KIOTA : CB_KIOTA + 1].bitcast(I32)
        rpat_v = cb32[:, CB_RPAT : CB_RPAT + 64].bitcast(I32)
        w1_v = cb32[:, CB_W1 : CB_W1 + 512].rearrange("p (a b c) -> p a b c", a=2, b=2)
        b1_v = cb32[:, CB_B1 : CB_B1 + 2]
        w2_v = cb32[:, CB_W2 : CB_W2 + 2]
        b2_v = cb32[0:1, CB_B2 : CB_B2 + 2]
        nones_v = cb16[:, :]

        # ================= Phase 1: conv A (n-partitioned) ==========
        # 16 groups of 512 positions; psum [128n, 4u x 256c]; relu alternates
        # ACT/Pool into fanall; resp row-sums on DVE (fp32).
        fanall = feat.tile([128, 64, 256], F16)
        resp_nb = small.tile([128, 64], F32)
        for g in range(16):
            imt = im_a[g // 8]
            ps = (cq_pool if g % 2 == 0 else cq2_pool).tile([128, 1024], F32, tag="cps")
            for u in range(4):
                o = 512 * (g % 8) + 128 * u
                nc.tensor.matmul(ps[:, 256 * u : 256 * (u + 1)],
                                 imt[0:28, o : o + 128], w27a_v,
                                 start=True, stop=True)
            dst = fanall[:, 4 * g : 4 * (g + 1), :]
            nc.scalar.activation(dst.rearrange("p a b -> p (a b)"), ps[:], AF.Relu)
            nc.sync.dma_start(
                AP(tensor=fa_t, offset=4 * g * 128 * COUT,
                   ap=[[COUT, 128], [128 * COUT, 4], [1, COUT]]),
                dst)
            nc.vector.tensor_reduce(
                resp_nb[:, 4 * g : 4 * g + 4], dst,
                axis=mybir.AxisListType.X, op=ALU.add)

        # ---- selection: packed-bits tree (unchanged from v1) ----
        rpk = small.tile([128, 64], F32)
        nc.vector.tensor_single_scalar(rpk[:].bitcast(I32),
                                       resp_nb[:].bitcast(I32), -8,
                                       ALU.bitwise_and)
        nc.vector.tensor_tensor(rpk[:].bitcast(I32), rpk[:].bitcast(I32),
                                rpat_v, ALU.bitwise_or)
        rv = rpk[:].rearrange("p (gr r) -> p gr r", gr=8)
        t1 = small.tile([128, 8, 4], F32)
        t2 = small.tile([128, 8, 2], F32)
        rowwin = small.tile([128, 8], F32)
        nc.vector.tensor_tensor(t1[:], rv[:, :, 0:4], rv[:, :, 4:8], ALU.max)
        nc.vector.tensor_tensor(t2[:], t1[:, :, 0:2], t1[:, :, 2:4], ALU.max)
        nc.vector.tensor_tensor(rowwin[:].rearrange("p (gr o) -> p gr o", o=1),
                                t2[:, :, 0:1], t2[:, :, 1:2], ALU.max)
        nc.sync.dma_start(
            AP(tensor=rvd, offset=0, ap=[[8, 128], [1, 8]]), rowwin[:])
        bpk = small.tile([128, 8], F32)
        nc.sync.dma_start(
            bpk[:], AP(tensor=rvd, offset=0, ap=[[1, 8], [64, 16], [8, 8]]))

        vmax8 = small.tile([128, 8], F32)
        vidx8 = small.tile([128, 8], U32)
        tmpu = small.tile([128, 1], I32)
        tmpv = small.tile([128, 1], I32)
        rowa_l = small.tile([128, 1], I32)
        rowa_g = small.tile([128, 1], I32)
        cola_l = small.tile([128, 1], I32)
        na_l = small.tile([128, 1], I32)
        nc.vector.max(vmax8[:], bpk[:])
        nc.vector.max_index(vidx8[:], vmax8[:], bpk[:])
        nc.vector.tensor_single_scalar(tmpu[:], vmax8[:, 0:1].bitcast(I32), 7,
                                       ALU.bitwise_and)
        nc.vector.tensor_single_scalar(tmpu[:], tmpu[:], -1, ALU.mult)
        nc.vector.tensor_single_scalar(tmpu[:], tmpu[:], 7, ALU.add)
        nc.vector.tensor_copy(tmpv[:].bitcast(U32), vidx8[:, 0:1])
        nc.vector.tensor_tensor(rowa_l[:], rowbl_v, tmpu[:], ALU.add)
        nc.vector.tensor_tensor(cola_l[:], colb_v, tmpv[:], ALU.add)
        nc.vector.tensor_single_scalar(na_l[:], rowa_l[:], 7, ALU.logical_shift_left)
        nc.vector.tensor_tensor(na_l[:], na_l[:], cola_l[:], ALU.add)
        nc.vector.tensor_tensor(rowa_g[:], rowa_l[:], row64_v, ALU.add)
        if dbg:
            nc.scalar.dma_start(na_dbg.ap(), na_l[:])

        # idx wrap for the descriptor gather: scr roundtrip, single read.
        na_i16 = small.tile([128, 1], I16)
        idxw = small.tile([128, 8], I16)
        nc.vector.tensor_copy(na_i16[:], na_l[:])
        nc.sync.dma_start(AP(tensor=scr, offset=0, ap=[[1, 128]]), na_i16[:])
        for gq in range(8):
            eng = nc.scalar if gq % 2 == 0 else nc.sync
            eng.dma_start(
                idxw[16 * gq : 16 * (gq + 1), :],
                AP(tensor=scr, offset=0, ap=[[1, 16], [16, 8]]))

        desc_i = small.tile([128, 2, 128], F16)   # [c, mb, k] (pre-scaled x2)

        # ============== Phase 2: conv B + scoring, fine-interleaved ========
        tmax = small.tile([128, 16, 8], F32)
        fbs = {}

        def convb_group(g):
            imt = im_b[g // 4]
            sl = slice(1024 * (g % 4), 1024 * (g % 4) + 1024)
            fb = fbt_pool.tile([128, 2, 1024], F16, tag="fb")
            fb2 = fbt_pool.tile([128, 2, 1024], F16, tag="fb2")
            for mb in range(2):
                ps = cq_pool.tile([128, 1024], F32, tag="cps")
                for q in range(2):
                    nc.tensor.matmul(ps[:, 512 * q : 512 * (q + 1)],
                                     w27_v[:, 128 * mb : 128 * (mb + 1)],
                                     imt[0:28, sl][:, 512 * q : 512 * (q + 1)],
                                     start=True, stop=True)
                if mb == 0:
                    nc.scalar.activation(fb[:, 0, :], ps[:], AF.Relu)
                else:
                    nc.vector.tensor_single_scalar(fb[:, 1, :], ps[:], 0.0, ALU.max)
            # squares: Pool takes whole even groups; odd groups split ACT/DVE
            if g % 2 == 0:
                nc.gpsimd.tensor_tensor(
                    fb2[:].rearrange("p a b -> p (a b)"),
                    fb[:].rearrange("p a b -> p (a b)"),
                    fb[:].rearrange("p a b -> p (a b)"), ALU.mult)
            else:
                nc.scalar.square(fb2[:, 0, :], fb[:, 0, :])
                nc.vector.tensor_tensor(fb2[:, 1, :], fb[:, 1, :], fb[:, 1, :],
                                        ALU.mult)
            fbs[g] = (fb, fb2)

        def score_chunk(c):
            fb, fb2 = fbs.pop(c)
            sp = cq2_pool.tile([128, 1024], F32, tag="cps")
            for s in range(2):
                sl2 = slice(512 * s, 512 * (s + 1))
                po = sp[:, sl2]
                nc.tensor.matmul(po, desc_i[:, 0, :], fb[:, 0, sl2], start=True, stop=False)
                nc.tensor.matmul(po, desc_i[:, 1, :], fb[:, 1, sl2], start=False, stop=False)
                nc.tensor.matmul(po, nones_v, fb2[:, 0, sl2], start=False, stop=False)
                nc.tensor.matmul(po, nones_v, fb2[:, 1, sl2], start=False, stop=True)
            nc.vector.max(tmax[:, c, :], sp[:])
            stage = fbt_pool.tile([128, 1024], F32, tag="stage")
            nc.scalar.activation(stage[:], sp[:], AF.Copy)
            nc.sync.dma_start(
                AP(tensor=scd, offset=c * 128 * 1024, ap=[[1024, 128], [1, 1024]]),
                stage[:])

        # PE order: 5 conv groups of lead, then interleave 1 group : 1 chunk.
        for g in range(5):
            convb_group(g)
        # descriptor gather sits here in the Pool stream: after the early
        # relus, before the steady-state ones.
        nc.gpsimd.dma_gather(
            desc_i[:], fa_t.ap(), idxw[:],
            num_idxs=128, num_idxs_reg=128, elem_size=COUT, transpose=True)
        if dbg:
            nc.gpsimd.dma_start(desc_dbg.ap(), desc_i[:])
        for g in range(5, 16):
            score_chunk(g - 5)
            convb_group(g)
        for c in range(11, 16):
            score_chunk(c)

        # ---- global winner over the 16 chunk-top-1s ----
        gmax = small.tile([128, 8], F32)
        gidx = small.tile([128, 8], U32)
        cstar = small.tile([128, 1], I32)
        sidx_i16 = small.tile([128, 1], I16)
        sidxw = small.tile([128, 8], I16)
        tview = tmax[:].rearrange("p a b -> p (a b)")[:, ::8]
        nc.vector.max(gmax[:], tview)
        nc.vector.max_index(gidx[:], gmax[:], tview)
        nc.vector.tensor_copy(cstar[:].bitcast(U32), gidx[:, 0:1])
        # scd row = c* * 128 + k
        scidx = small.tile([128, 1], I32)
        nc.vector.tensor_single_scalar(scidx[:], cstar[:], 7, ALU.logical_shift_left)
        nc.vector.tensor_tensor(scidx[:], scidx[:], kiota_v, ALU.add)
        nc.vector.tensor_copy(sidx_i16[:], scidx[:])
        nc.sync.dma_start(AP(tensor=scr2, offset=0, ap=[[1, 128]]), sidx_i16[:])
        for gq in range(8):
            eng = nc.scalar if gq % 2 == 0 else nc.sync
            eng.dma_start(
                sidxw[16 * gq : 16 * (gq + 1), :],
                AP(tensor=scr2, offset=0, ap=[[1, 16], [16, 8]]))

        wchunk = small.tile([128, 1, 1024], F32)
        nc.gpsimd.dma_gather(
            wchunk[:], scd.ap(), sidxw[:],
            num_idxs=128, num_idxs_reg=128, elem_size=1024, transpose=False)
        fidx = small.tile([128, 8], U32)
        nc.vector.max_index(fidx[:], gmax[:], wchunk[:, 0, :])
        within = small.tile([128, 1], I32)
        nb = small.tile([128, 1], I32)
        nc.vector.tensor_copy(within[:].bitcast(U32), fidx[:, 0:1])
        nc.vector.tensor_single_scalar(nb[:], cstar[:], 10, ALU.logical_shift_left)
        nc.vector.tensor_tensor(nb[:], nb[:], within[:], ALU.add)
        if dbg:
            nc.scalar.dma_start(nb_dbg.ap(), nb[:])

        # ---- displacements ----
        rowb_t = small.tile([128, 1], I32)
        colb_t = small.tile([128, 1], I32)
        di_t = small.tile([128, 1], I32)
        d_f = small.tile([128, 2], F32)
        nc.vector.tensor_single_scalar(rowb_t[:], nb[:], 7, ALU.logical_shift_right)
        nc.vector.tensor_single_scalar(colb_t[:], nb[:], 127, ALU.bitwise_and)
        nc.vector.tensor_tensor(di_t[:], rowb_t[:], rowa_g[:], ALU.subtract)
        nc.vector.tensor_copy(d_f[:, 0:1], di_t[:])
        nc.vector.tensor_tensor(di_t[:], cola_l[:], colb_t[:], ALU.subtract)
        nc.vector.tensor_copy(d_f[:, 1:2], di_t[:])
        if dbg:
            nc.scalar.dma_start(drow_dbg.ap(), d_f[:])

        # ---- Exchange: AllGather displacement halves in the pair ----
        ex_in = dram.tile([128, 2], F32)
        ex_out = dram.tile([2, 128, 2], F32)
        nc.sync.dma_start(ex_in[:], d_f[:])
        nc.gpsimd.collective_compute(
            "AllGather", ALU.bypass,
            replica_groups=[[0, 1], [2, 3], [4, 5], [6, 7]],
            ins=[ex_in.opt()], outs=[ex_out.opt()])
        d_all = small.tile([128, 2, 2], F32)   # [k, half, rc]
        nc.sync.dma_start(d_all[:], ex_out[:].rearrange("r p c -> p r c"))

        # ---- MLPs ----
        out_sb = small.tile([1, 2], F32)
        hid = small.tile([128, 1], F32)
        b1s = small.tile([128, 2], F32)
        b2s = small.tile([1, 2], F32)
        nc.vector.tensor_copy(b1s[:], b1_v)
        nc.vector.tensor_copy(b2s[:], b2_v)
        for rc in range(2):
            hp = cq2_pool.tile([128, 1024], F32, tag="cps")
            for half in range(2):
                nc.tensor.matmul(hp[:, 0:1], w1_v[:, rc, half, :],
                                 d_all[:, half, rc : rc + 1],
                                 start=(half == 0), stop=(half == 1))
            nc.scalar.activation(hid[:], hp[:, 0:1], AF.Relu, bias=b1s[:, rc : rc + 1])
            op = cq2_pool.tile([128, 1024], F32, tag="cps")
            nc.tensor.matmul(op[:1, 0:1], hid[:], w2_v[:, rc : rc + 1],
                             start=True, stop=True)
            nc.scalar.activation(out_sb[:, rc : rc + 1], op[:1, 0:1], AF.Identity,
                                 bias=b2s[:, rc : rc + 1])
        nc.sync.dma_start(out.ap(), out_sb[:])

    nc.compile()
    return nc


_NC_CACHE = {}


def _get_nc(dbg=False):
    if dbg not in _NC_CACHE:
        _NC_CACHE[dbg] = build_kernel(dbg=dbg)
    return _NC_CACHE[dbg]


def _host_inputs(inputs):
    xA = np.asarray(inputs["xA"], np.float32)
    xB = np.asarray(inputs["xB"], np.float32)
    Wc = np.asarray(inputs["Wconv"], dtype=np.float32)
    bc = np.asarray(inputs["bconv"], dtype=np.float32)

    blob32 = np.zeros((128, CB_COLS), dtype=np.float32)
    bi = blob32.view(np.int32)
    wblob = np.zeros((32, 512), dtype=np.float32)
    wblob[:, 0:256] = _prep_w27(Wc, bc, 4.0)
    wblob[:, 256:512] = _prep_w27(Wc, bc, 2.0)
    p = np.arange(128)
    bi[:, CB_ROWBL] = (8 * (p // 16)).astype(np.int32)
    bi[:, CB_COLB] = (8 * (p % 16)).astype(np.int32)
    bi[:, CB_KIOTA] = p.astype(np.int32)
    bi[:, CB_RPAT : CB_RPAT + 64] = np.broadcast_to(
        7 - (np.arange(64, dtype=np.int32) % 8), (128, 64))
    w1 = np.stack([
        np.asarray(inputs["W1r"], np.float32).reshape(2, 128, 128),
        np.asarray(inputs["W1c"], np.float32).reshape(2, 128, 128),
    ])  # [rc, half, p, j]
    blob32[:, CB_W1 : CB_W1 + 512] = w1.transpose(2, 0, 1, 3).reshape(128, 512)
    blob32[:, CB_B1 : CB_B1 + 2] = np.stack(
        [np.asarray(inputs["b1r"], np.float32), np.asarray(inputs["b1c"], np.float32)], 1)
    blob32[:, CB_W2 : CB_W2 + 2] = np.concatenate(
        [np.asarray(inputs["W2r"], np.float32), np.asarray(inputs["W2c"], np.float32)], 1)
    blob32[0, CB_B2 : CB_B2 + 2] = [float(np.asarray(inputs["b2r"])[0]),
                                    float(np.asarray(inputs["b2c"])[0])]
    blob16 = -np.ones((128, 128), dtype=np.float16)

    in_maps = []
    for c in range(NCORES):
        b, par = c // 2, c % 2
        m = dict(blob32=blob32, blob16=blob16, wblob=wblob)
        m["xa"] = _prep_planes(xA[b], 64 * par, 64)
        m["xb"] = _prep_planes(xB[b], 0, 128)
        bl = blob32.copy()
        bl.view(np.int32)[:, CB_ROW64] = 64 * par
        m["blob32"] = bl
        in_maps.append(m)
    return in_maps


def kernel(**inputs):
    nc = _get_nc(dbg=False)
    in_maps = _host_inputs(inputs)
    res = bass_utils.run_bass_kernel_spmd(nc, in_maps, core_ids=list(range(NCORES)))
    return np.concatenate([res.results[2 * b]["out"] for b in range(B)], axis=0)


def kernel_dbg(**inputs):
    nc = _get_nc(dbg=True)
    in_maps = _host_inputs(inputs)
    res = bass_utils.run_bass_kernel_spmd(nc, in_maps, core_ids=list(range(NCORES)))
    out = np.concatenate([res.results[2 * b]["out"] for b in range(B)], axis=0)
    return out, res.results


# revision 13
# speedup vs baseline: 1.0501x; 1.0501x over previous
"""DeepStitch Trainium2 Bass kernel (8-core, full-B replication), v2.

Core 2b+par owns image b and descriptor half par.  Per core:
  conv A on its own row-half, n-PARTITIONED into an SBUF-resident feature
  bank fanall [128, 64, 256] fp16 (partition = n%128, rank = n//128) with
  relu alternating ACT/Pool; resp row-sums on DVE (fp32, exact vs ref) ->
  packed-bits selection tree (unchanged) -> 128 descriptor indices ->
  SBUF-source dma_gather pulls descriptors c-partitioned [c, mb, k] ->
  conv B (full image, 16 groups of 1024 positions) fine-interleaved with
  16 scoring chunks on PE so the tensor engine never stalls; relu mb0/mb1
  split ACT/Pool, squares on DVE (fp16 2x); per-chunk top-1 via a single
  DVE Max, with the chunk psum spilled fp32 to DRAM in parallel; global
  winner = Max/MaxIndex over the 16 chunk maxima, then one dma_gather
  re-fetches each k's winning 1024-chunk and one MaxIndex recovers the
  exact argmin index -> displacements -> single pairwise AllGather ->
  row/col MLPs (host reads even cores' out).

Conv bias rides as a 28th im2col row; conv-A weights are x4 and conv-B x2
so gathered descriptors arrive pre-scaled (desc.fb - |fb|^2 = 4*(2ab-b^2),
argmin-equivalent).  Conv matmuls in float32r; scoring in fp16 (index-
exact vs the fp32 reference).  All constants ride in two host-packed blob
DMAs; im2col loads are single 28-descriptor DMAs from host-prepped
stride-2 tap planes.
"""

import sys

for _p in ("/opt/trn_rl_repo",):
    if _p not in sys.path:
        sys.path.insert(0, _p)

import numpy as np

import concourse.bacc as bacc
import concourse.bass as bass
import concourse.mybir as mybir
import concourse.tile as tile
import concourse.bass_utils as bass_utils
from concourse.bass import AP
from contextlib import ExitStack

F32 = mybir.dt.float32
F32R = mybir.dt.float32r
F16 = mybir.dt.float16
I16 = mybir.dt.int16
I32 = mybir.dt.int32
U32 = mybir.dt.uint32
AF = mybir.ActivationFunctionType
ALU = mybir.AluOpType

B = 4
NCORES = 8
CIN = 3
COUT = 256
H = W = 128
NH = 8192           # half-image positions (conv-A share)
NF = 16384          # full-image positions (conv-B)

_DYS = {0: [0, 2], 1: [1]}

# ---- const blob layout (f32 columns) ----
CB_W27A = 0          # [32, 256] f32r
CB_W27 = 256         # [32, 256] f32r
CB_ROWBL = 512       # [128, 1] i32
CB_COLB = 513        # [128, 1] i32
CB_ROW64 = 514       # [128, 1] i32
CB_KIOTA = 515       # [128, 1] i32 (= partition index k)
CB_RPAT = 516        # [128, 64] i32
CB_W1 = 580          # [128, 2, 2, 128] f32
CB_B1 = 1092         # [128, 2] f32
CB_W2 = 1094         # [128, 2] f32
CB_B2 = 1096         # [1, 2] f32 (partition 0)
CB_IOTA16 = 1098     # [128, 16] f32
CB_COLS = 1114


def _tap_order():
    taps = []
    for pr in (0, 1):
        for pc in (0, 1):
            for c in range(CIN):
                for dy in _DYS[pr]:
                    for dx in _DYS[pc]:
                        taps.append((c, dy, dx))
    assert len(taps) == 27
    return taps


TAPS = _tap_order()


def _prep_planes(x, r0, nrows):
    """[3,256,256] -> per-tap stride-2 planes [28, nrows*128]; row 27 = ones
    (bias row, pairs with the bias row of the weight matrix)."""
    xp = np.zeros((CIN, 259, 259), dtype=np.float32)
    xp[:, 1:257, 1:257] = x
    out = np.empty((28, nrows, 128), dtype=np.float32)
    for t, (c, dy, dx) in enumerate(TAPS):
        sub = xp[c, dy::2, dx::2]
        out[t] = sub[r0 : r0 + nrows, :128]
    out[27] = 1.0
    return out.reshape(28, nrows * 128)


def _prep_w27(Wconv, bconv, scale):
    w = np.zeros((32, COUT), dtype=np.float32)
    for i, (c, dy, dx) in enumerate(TAPS):
        w[i] = scale * Wconv[:, c, dy, dx]
    w[27] = scale * bconv
    return w


def build_kernel(dbg=False):
    nc = bacc.Bacc("TRN2", target_bir_lowering=False, debug=False,
                   num_devices=NCORES)

    blob32 = nc.dram_tensor("blob32", [128, CB_COLS], F32, kind="ExternalInput")
    blob16 = nc.dram_tensor("blob16", [128, 128], F16, kind="ExternalInput")
    wblob = nc.dram_tensor("wblob", [32, 512], F32R, kind="ExternalInput")
    xa = nc.dram_tensor("xa", [28, NH], F32R, kind="ExternalInput")
    xb = nc.dram_tensor("xb", [28, NF], F32R, kind="ExternalInput")
    out = nc.dram_tensor("out", [1, 2], F32, kind="ExternalOutput")
    scr = nc.dram_tensor("scr", [128], I16, kind="Internal")
    rvd = nc.dram_tensor("rvd", [2048], F32, kind="Internal")
    fa_t = nc.dram_tensor("fa_t", [NH, COUT], F16, kind="Internal")

    if dbg:
        na_dbg = nc.dram_tensor("na_dbg", [128, 1], I32, kind="ExternalOutput")
        desc_dbg = nc.dram_tensor("desc_dbg", [128, 2, 128], F16, kind="ExternalOutput")
        nb_dbg = nc.dram_tensor("nb_dbg", [128, 1], I32, kind="ExternalOutput")
        drow_dbg = nc.dram_tensor("drow_dbg", [128, 2], F32, kind="ExternalOutput")

    with tile.TileContext(nc) as tc, ExitStack() as ctx:
        const = ctx.enter_context(tc.tile_pool(name="const", bufs=1))
        small = ctx.enter_context(tc.tile_pool(name="small", bufs=1))
        feat = ctx.enter_context(tc.tile_pool(name="feat", bufs=1))
        im_pool = ctx.enter_context(tc.tile_pool(name="im", bufs=4))
        fbt_pool = ctx.enter_context(tc.tile_pool(name="fbt", bufs=8))
        dram = ctx.enter_context(tc.tile_pool(name="dram", bufs=1, space="DRAM"))
        cq_pool = ctx.enter_context(tc.tile_pool(name="cq", bufs=2, space="PSUM"))
        cq2_pool = ctx.enter_context(tc.tile_pool(name="cq2", bufs=2, space="PSUM"))

        # ---- head: all input DMAs on the SP queue, deps-free, in use order ----
        cb32 = const.tile([128, CB_COLS], F32, tag="cb32")
        cb16 = const.tile([128, 128], F16, tag="cb16")
        cwb = const.tile([32, 512], F32R, tag="cwb")
        nc.sync.dma_start(cwb[:], wblob.ap())
        im_a = []
        for ha in range(2):
            t = im_pool.tile([32, NH // 2], F32R, tag="im")
            nc.sync.dma_start(
                t[0:28, :], AP(tensor=xa, offset=ha * (NH // 2),
                               ap=[[NH, 28], [1, NH // 2]]))
            im_a.append(t)
        im_b = [None] * 4

        def load_imb(hb):
            t = im_pool.tile([32, 4096], F32R, tag="im")
            nc.sync.dma_start(
                t[0:28, :], AP(tensor=xb, offset=hb * 4096,
                               ap=[[NF, 28], [1, 4096]]))
            im_b[hb] = t

        load_imb(0)
        load_imb(1)
        nc.sync.dma_start(cb32[:], blob32.ap())
        nc.sync.dma_start(cb16[:], blob16.ap())

        # const views
        w27a_v = cwb[0:28, 0:256]
        w27_v = cwb[0:28, 256:512]
        rowbl_v = cb32[:, CB_ROWBL : CB_ROWBL + 1].bitcast(I32)
        colb_v = cb32[:, CB_COLB : CB_COLB + 1].bitcast(I32)
        row64_v = cb32[:, CB_ROW64 : CB_ROW64 + 1].bitcast(I32)
        kiota_v = cb32[:, CB_KIOTA : CB_KIOTA + 1].bitcast(I32)
        rpat_v = cb32[:, CB_RPAT : CB_RPAT + 64].bitcast(I32)
        w1_v = cb32[:, CB_W1 : CB_W1 + 512].rearrange("p (a b c) -> p a b c", a=2, b=2)
        b1_v = cb32[:, CB_B1 : CB_B1 + 2]
        w2_v = cb32[:, CB_W2 : CB_W2 + 2]
        b2_v = cb32[0:1, CB_B2 : CB_B2 + 2]
        iota16_v = cb32[:, CB_IOTA16 : CB_IOTA16 + 16]
        nones_v = cb16[:, :]

        # ================= Phase 1: conv A (n-partitioned) ==========
        # 16 groups of 512 positions; psum [128n, 4u x 256c]; relu alternates
        # ACT/Pool into fanall; resp row-sums on DVE (fp32).
        fanall = feat.tile([128, 64, 256], F16)
        resp_nb = small.tile([128, 64], F32)
        for g in range(16):
            if g == 9:
                load_imb(2)   # reuses slot of im_a[0] (free after group 7)
            imt = im_a[g // 8]
            ps = (cq_pool if g % 2 == 0 else cq2_pool).tile([128, 1024], F32, tag="cps")
            for u in range(4):
                o = 512 * (g % 8) + 128 * u
                nc.tensor.matmul(ps[:, 256 * u : 256 * (u + 1)],
                                 imt[0:28, o : o + 128], w27a_v,
                                 start=True, stop=True)
            dst = fanall[:, 4 * g : 4 * (g + 1), :]
            nc.scalar.activation(dst.rearrange("p a b -> p (a b)"), ps[:], AF.Relu)
            nc.sync.dma_start(
                AP(tensor=fa_t, offset=4 * g * 128 * COUT,
                   ap=[[COUT, 128], [128 * COUT, 4], [1, COUT]]),
                dst)
            nc.vector.tensor_reduce(
                resp_nb[:, 4 * g : 4 * g + 4], dst,
                axis=mybir.AxisListType.X, op=ALU.add)

        load_imb(3)   # reuses slot of im_a[1] (free after group 15)

        # ---- selection: packed-bits tree (unchanged from v1) ----
        rpk = small.tile([128, 64], F32)
        nc.vector.tensor_single_scalar(rpk[:].bitcast(I32),
                                       resp_nb[:].bitcast(I32), -8,
                                       ALU.bitwise_and)
        nc.vector.tensor_tensor(rpk[:].bitcast(I32), rpk[:].bitcast(I32),
                                rpat_v, ALU.bitwise_or)
        rv = rpk[:].rearrange("p (gr r) -> p gr r", gr=8)
        t1 = small.tile([128, 8, 4], F32)
        t2 = small.tile([128, 8, 2], F32)
        rowwin = small.tile([128, 8], F32)
        nc.vector.tensor_tensor(t1[:], rv[:, :, 0:4], rv[:, :, 4:8], ALU.max)
        nc.vector.tensor_tensor(t2[:], t1[:, :, 0:2], t1[:, :, 2:4], ALU.max)
        nc.vector.tensor_tensor(rowwin[:].rearrange("p (gr o) -> p gr o", o=1),
                                t2[:, :, 0:1], t2[:, :, 1:2], ALU.max)
        nc.sync.dma_start(
            AP(tensor=rvd, offset=0, ap=[[8, 128], [1, 8]]), rowwin[:])
        bpk = small.tile([128, 8], F32)
        nc.sync.dma_start(
            bpk[:], AP(tensor=rvd, offset=0, ap=[[1, 8], [64, 16], [8, 8]]))

        vmax8 = small.tile([128, 8], F32)
        vidx8 = small.tile([128, 8], U32)
        tmpu = small.tile([128, 1], I32)
        tmpv = small.tile([128, 1], I32)
        rowa_l = small.tile([128, 1], I32)
        rowa_g = small.tile([128, 1], I32)
        cola_l = small.tile([128, 1], I32)
        na_l = small.tile([128, 1], I32)
        nc.vector.max(vmax8[:], bpk[:])
        nc.vector.max_index(vidx8[:], vmax8[:], bpk[:])
        nc.vector.tensor_single_scalar(tmpu[:], vmax8[:, 0:1].bitcast(I32), 7,
                                       ALU.bitwise_and)
        nc.vector.tensor_single_scalar(tmpu[:], tmpu[:], -1, ALU.mult)
        nc.vector.tensor_single_scalar(tmpu[:], tmpu[:], 7, ALU.add)
        nc.vector.tensor_copy(tmpv[:].bitcast(U32), vidx8[:, 0:1])
        nc.vector.tensor_tensor(rowa_l[:], rowbl_v, tmpu[:], ALU.add)
        nc.vector.tensor_tensor(cola_l[:], colb_v, tmpv[:], ALU.add)
        nc.vector.tensor_single_scalar(na_l[:], rowa_l[:], 7, ALU.logical_shift_left)
        nc.vector.tensor_tensor(na_l[:], na_l[:], cola_l[:], ALU.add)
        nc.vector.tensor_tensor(rowa_g[:], rowa_l[:], row64_v, ALU.add)
        if dbg:
            nc.scalar.dma_start(na_dbg.ap(), na_l[:])

        # idx wrap for the descriptor gather: scr roundtrip, single read.
        na_i16 = small.tile([128, 1], I16)
        idxw = small.tile([128, 8], I16)
        nc.vector.tensor_copy(na_i16[:], na_l[:])
        nc.sync.dma_start(AP(tensor=scr, offset=0, ap=[[1, 128]]), na_i16[:])
        for gq in range(8):
            eng = nc.scalar if gq % 2 == 0 else nc.sync
            eng.dma_start(
                idxw[16 * gq : 16 * (gq + 1), :],
                AP(tensor=scr, offset=0, ap=[[1, 16], [16, 8]]))

        desc_i = small.tile([128, 2, 128], F16)   # [c, mb, k] (pre-scaled x2)

        # ============== Phase 2: conv B + scoring, fine-interleaved ========
        tmax = small.tile([128, 16, 8], F32)
        tidx = small.tile([128, 16, 8], U32)
        fbs = {}
        NLEAD = 6

        def convb_group(g):
            imt = im_b[g // 4]
            sl = slice(1024 * (g % 4), 1024 * (g % 4) + 1024)
            fb = fbt_pool.tile([128, 2, 1024], F16, tag="fb")
            fb2 = fbt_pool.tile([128, 2, 1024], F16, tag="fb2")
            for mb in range(2):
                ps = cq_pool.tile([128, 1024], F32, tag="cps")
                for q in range(2):
                    nc.tensor.matmul(ps[:, 512 * q : 512 * (q + 1)],
                                     w27_v[:, 128 * mb : 128 * (mb + 1)],
                                     imt[0:28, sl][:, 512 * q : 512 * (q + 1)],
                                     start=True, stop=True)
                nc.scalar.activation(fb[:, mb, :], ps[:], AF.Relu)
            # squares: leads on ACT/DVE (Pool must reach the desc gather
            # first); steady state: Pool takes mb0, ACT/DVE alternate mb1.
            if g < NLEAD:
                nc.scalar.square(fb2[:, 0, :], fb[:, 0, :])
                nc.vector.tensor_tensor(fb2[:, 1, :], fb[:, 1, :], fb[:, 1, :],
                                        ALU.mult)
            else:
                nc.gpsimd.tensor_tensor(fb2[:, 0, :], fb[:, 0, :], fb[:, 0, :],
                                        ALU.mult)
                if g % 2 == 0:
                    nc.vector.tensor_tensor(fb2[:, 1, :], fb[:, 1, :],
                                            fb[:, 1, :], ALU.mult)
                else:
                    nc.scalar.square(fb2[:, 1, :], fb[:, 1, :])
            fbs[g] = (fb, fb2)

        def score_chunk(c):
            fb, fb2 = fbs.pop(c)
            sp = cq2_pool.tile([128, 1024], F32, tag="cps")
            for s in range(2):
                sl2 = slice(512 * s, 512 * (s + 1))
                po = sp[:, sl2]
                nc.tensor.matmul(po, desc_i[:, 0, :], fb[:, 0, sl2], start=True, stop=False)
                nc.tensor.matmul(po, desc_i[:, 1, :], fb[:, 1, sl2], start=False, stop=False)
                nc.tensor.matmul(po, nones_v, fb2[:, 0, sl2], start=False, stop=False)
                nc.tensor.matmul(po, nones_v, fb2[:, 1, sl2], start=False, stop=True)
            nc.vector.max(tmax[:, c, :], sp[:])
            nc.vector.max_index(tidx[:, c, :], tmax[:, c, :], sp[:])

        # Pool's first instruction is the descriptor gather; PE leads with
        # NLEAD conv groups, then interleaves 1 chunk : 1 group.
        nc.gpsimd.dma_gather(
            desc_i[:], fa_t.ap(), idxw[:],
            num_idxs=128, num_idxs_reg=128, elem_size=COUT, transpose=True)
        if dbg:
            nc.gpsimd.dma_start(desc_dbg.ap(), desc_i[:])
        for g in range(NLEAD):
            convb_group(g)
        for g in range(NLEAD, 16):
            score_chunk(g - NLEAD)
            convb_group(g)
        for c in range(16 - NLEAD, 16):
            score_chunk(c)

        # ---- global winner over the 16 chunk-top-1s ----
        gmx8 = small.tile([128, 8], F32)
        gix8 = small.tile([128, 8], U32)
        cstar = small.tile([128, 1], U32)
        cstarf = small.tile([128, 1], F32)
        mask16 = small.tile([128, 16], F32)
        locf = small.tile([128, 1], F32)
        locu = small.tile([128, 1], U32)
        nb = small.tile([128, 1], I32)
        tview = tmax[:].rearrange("p a b -> p (a b)")[:, ::8]
        iview = tidx[:].rearrange("p a b -> p (a b)")[:, ::8]
        nc.vector.max(gmx8[:], tview)
        nc.vector.max_index(gix8[:], gmx8[:], tview)
        nc.vector.tensor_copy(cstar[:], gix8[:, 0:1])
        nc.vector.tensor_copy(cstarf[:], cstar[:])
        nc.vector.tensor_scalar(mask16[:], iota16_v, cstarf[:], None, ALU.is_equal)
        nc.vector.tensor_tensor(mask16[:], mask16[:], iview, ALU.mult)
        nc.vector.tensor_reduce(locf[:], mask16[:], axis=mybir.AxisListType.X, op=ALU.add)
        nc.vector.tensor_copy(locu[:], locf[:])
        nc.vector.tensor_single_scalar(cstar[:], cstar[:], 10, ALU.logical_shift_left)
        nc.vector.tensor_tensor(nb[:].bitcast(U32), cstar[:], locu[:], ALU.add)
        if dbg:
            nc.scalar.dma_start(nb_dbg.ap(), nb[:])

        # ---- displacements ----
        rowb_t = small.tile([128, 1], I32)
        colb_t = small.tile([128, 1], I32)
        di_t = small.tile([128, 1], I32)
        d_f = small.tile([128, 2], F32)
        nc.vector.tensor_single_scalar(rowb_t[:], nb[:], 7, ALU.logical_shift_right)
        nc.vector.tensor_single_scalar(colb_t[:], nb[:], 127, ALU.bitwise_and)
        nc.vector.tensor_tensor(di_t[:], rowb_t[:], rowa_g[:], ALU.subtract)
        nc.vector.tensor_copy(d_f[:, 0:1], di_t[:])
        nc.vector.tensor_tensor(di_t[:], cola_l[:], colb_t[:], ALU.subtract)
        nc.vector.tensor_copy(d_f[:, 1:2], di_t[:])
        if dbg:
            nc.scalar.dma_start(drow_dbg.ap(), d_f[:])

        # ---- Exchange: AllGather displacement halves in the pair ----
        ex_in = dram.tile([128, 2], F32)
        ex_out = dram.tile([2, 128, 2], F32)
        nc.sync.dma_start(ex_in[:], d_f[:])
        nc.gpsimd.collective_compute(
            "AllGather", ALU.bypass,
            replica_groups=[[0, 1], [2, 3], [4, 5], [6, 7]],
            ins=[ex_in.opt()], outs=[ex_out.opt()])
        d_all = small.tile([128, 2, 2], F32)   # [k, half, rc]
        nc.sync.dma_start(d_all[:], ex_out[:].rearrange("r p c -> p r c"))

        # ---- MLPs ----
        out_sb = small.tile([1, 2], F32)
        hid = small.tile([128, 1], F32)
        b1s = small.tile([128, 2], F32)
        b2s = small.tile([1, 2], F32)
        nc.vector.tensor_copy(b1s[:], b1_v)
        nc.vector.tensor_copy(b2s[:], b2_v)
        for rc in range(2):
            hp = cq2_pool.tile([128, 1024], F32, tag="cps")
            for half in range(2):
                nc.tensor.matmul(hp[:, 0:1], w1_v[:, rc, half, :],
                                 d_all[:, half, rc : rc + 1],
                                 start=(half == 0), stop=(half == 1))
            nc.scalar.activation(hid[:], hp[:, 0:1], AF.Relu, bias=b1s[:, rc : rc + 1])
            op = cq2_pool.tile([128, 1024], F32, tag="cps")
            nc.tensor.matmul(op[:1, 0:1], hid[:], w2_v[:, rc : rc + 1],
                             start=True, stop=True)
            nc.scalar.activation(out_sb[:, rc : rc + 1], op[:1, 0:1], AF.Identity,
                                 bias=b2s[:, rc : rc + 1])
        nc.sync.dma_start(out.ap(), out_sb[:])

    nc.compile()
    return nc


_NC_CACHE = {}


def _get_nc(dbg=False):
    if dbg not in _NC_CACHE:
        _NC_CACHE[dbg] = build_kernel(dbg=dbg)
    return _NC_CACHE[dbg]


def _host_inputs(inputs):
    xA = np.asarray(inputs["xA"], np.float32)
    xB = np.asarray(inputs["xB"], np.float32)
    Wc = np.asarray(inputs["Wconv"], dtype=np.float32)
    bc = np.asarray(inputs["bconv"], dtype=np.float32)

    blob32 = np.zeros((128, CB_COLS), dtype=np.float32)
    bi = blob32.view(np.int32)
    wblob = np.zeros((32, 512), dtype=np.float32)
    wblob[:, 0:256] = _prep_w27(Wc, bc, 4.0)
    wblob[:, 256:512] = _prep_w27(Wc, bc, 2.0)
    p = np.arange(128)
    bi[:, CB_ROWBL] = (8 * (p // 16)).astype(np.int32)
    bi[:, CB_COLB] = (8 * (p % 16)).astype(np.int32)
    bi[:, CB_KIOTA] = p.astype(np.int32)
    bi[:, CB_RPAT : CB_RPAT + 64] = np.broadcast_to(
        7 - (np.arange(64, dtype=np.int32) % 8), (128, 64))
    w1 = np.stack([
        np.asarray(inputs["W1r"], np.float32).reshape(2, 128, 128),
        np.asarray(inputs["W1c"], np.float32).reshape(2, 128, 128),
    ])  # [rc, half, p, j]
    blob32[:, CB_W1 : CB_W1 + 512] = w1.transpose(2, 0, 1, 3).reshape(128, 512)
    blob32[:, CB_B1 : CB_B1 + 2] = np.stack(
        [np.asarray(inputs["b1r"], np.float32), np.asarray(inputs["b1c"], np.float32)], 1)
    blob32[:, CB_W2 : CB_W2 + 2] = np.concatenate(
        [np.asarray(inputs["W2r"], np.float32), np.asarray(inputs["W2c"], np.float32)], 1)
    blob32[0, CB_B2 : CB_B2 + 2] = [float(np.asarray(inputs["b2r"])[0]),
                                    float(np.asarray(inputs["b2c"])[0])]
    blob32[:, CB_IOTA16 : CB_IOTA16 + 16] = np.arange(16, dtype=np.float32)[None, :]
    blob16 = -np.ones((128, 128), dtype=np.float16)

    in_maps = []
    for c in range(NCORES):
        b, par = c // 2, c % 2
        m = dict(blob32=blob32, blob16=blob16, wblob=wblob)
        m["xa"] = _prep_planes(xA[b], 64 * par, 64)
        m["xb"] = _prep_planes(xB[b], 0, 128)
        bl = blob32.copy()
        bl.view(np.int32)[:, CB_ROW64] = 64 * par
        m["blob32"] = bl
        in_maps.append(m)
    return in_maps


def kernel(**inputs):
    nc = _get_nc(dbg=False)
    in_maps = _host_inputs(inputs)
    res = bass_utils.run_bass_kernel_spmd(nc, in_maps, core_ids=list(range(NCORES)))
    return np.concatenate([res.results[2 * b]["out"] for b in range(B)], axis=0)


def kernel_dbg(**inputs):
    nc = _get_nc(dbg=True)
    in_maps = _host_inputs(inputs)
    res = bass_utils.run_bass_kernel_spmd(nc, in_maps, core_ids=list(range(NCORES)))
    out = np.concatenate([res.results[2 * b]["out"] for b in range(B)], axis=0)
    return out, res.results


# revision 15
# speedup vs baseline: 1.1203x; 1.0668x over previous
"""DeepStitch Trainium2 Bass kernel (8-core, full-B replication), v2.

Core 2b+par owns image b and descriptor half par.  Per core:
  conv A on its own row-half, n-PARTITIONED into an SBUF-resident feature
  bank fanall [128, 64, 256] fp16 (partition = n%128, rank = n//128) with
  relu alternating ACT/Pool; resp row-sums on DVE (fp32, exact vs ref) ->
  packed-bits selection tree (unchanged) -> 128 descriptor indices ->
  SBUF-source dma_gather pulls descriptors c-partitioned [c, mb, k] ->
  conv B (full image, 16 groups of 1024 positions) fine-interleaved with
  16 scoring chunks on PE so the tensor engine never stalls; relu mb0/mb1
  split ACT/Pool, squares on DVE (fp16 2x); per-chunk top-1 via a single
  DVE Max, with the chunk psum spilled fp32 to DRAM in parallel; global
  winner = Max/MaxIndex over the 16 chunk maxima, then one dma_gather
  re-fetches each k's winning 1024-chunk and one MaxIndex recovers the
  exact argmin index -> displacements -> single pairwise AllGather ->
  row/col MLPs (host reads even cores' out).

Conv bias rides as a 28th im2col row; conv-A weights are x4 and conv-B x2
so gathered descriptors arrive pre-scaled (desc.fb - |fb|^2 = 4*(2ab-b^2),
argmin-equivalent).  Conv matmuls in float32r; scoring in fp16 (index-
exact vs the fp32 reference).  All constants ride in two host-packed blob
DMAs; im2col loads are single 28-descriptor DMAs from host-prepped
stride-2 tap planes.
"""

import sys

for _p in ("/opt/trn_rl_repo",):
    if _p not in sys.path:
        sys.path.insert(0, _p)

import numpy as np

import concourse.bacc as bacc
import concourse.bass as bass
import concourse.mybir as mybir
import concourse.tile as tile
import concourse.bass_utils as bass_utils
from concourse.bass import AP
from contextlib import ExitStack

F32 = mybir.dt.float32
F32R = mybir.dt.float32r
F16 = mybir.dt.float16
I16 = mybir.dt.int16
I32 = mybir.dt.int32
U32 = mybir.dt.uint32
AF = mybir.ActivationFunctionType
ALU = mybir.AluOpType

B = 4
NCORES = 8
CIN = 3
COUT = 256
H = W = 128
NH = 8192           # half-image positions (conv-A share)
NF = 16384          # full-image positions (conv-B)

_DYS = {0: [0, 2], 1: [1]}

# ---- const blob layout (f32 columns) ----
CB_W27A = 0          # [32, 256] f32r
CB_W27 = 256         # [32, 256] f32r
CB_ROWBL = 512       # [128, 1] i32
CB_COLB = 513        # [128, 1] i32
CB_ROW64 = 514       # [128, 1] i32
CB_KIOTA = 515       # [128, 1] i32 (= partition index k)
CB_RPAT = 516        # [128, 64] i32
CB_W1 = 580          # [128, 2, 2, 128] f32
CB_B1 = 1092         # [128, 2] f32
CB_W2 = 1094         # [128, 2] f32
CB_B2 = 1096         # [1, 2] f32 (partition 0)
CB_IOTA16 = 1098     # [128, 16] f32
CB_COLS = 1114


def _tap_order():
    taps = []
    for pr in (0, 1):
        for pc in (0, 1):
            for c in range(CIN):
                for dy in _DYS[pr]:
                    for dx in _DYS[pc]:
                        taps.append((c, dy, dx))
    assert len(taps) == 27
    return taps


TAPS = _tap_order()


def _prep_planes(x, r0, nrows):
    """[3,256,256] -> per-tap stride-2 planes [28, nrows*128]; row 27 = ones
    (bias row, pairs with the bias row of the weight matrix)."""
    xp = np.zeros((CIN, 259, 259), dtype=np.float32)
    xp[:, 1:257, 1:257] = x
    out = np.empty((28, nrows, 128), dtype=np.float32)
    for t, (c, dy, dx) in enumerate(TAPS):
        sub = xp[c, dy::2, dx::2]
        out[t] = sub[r0 : r0 + nrows, :128]
    out[27] = 1.0
    return out.reshape(28, nrows * 128)


def _prep_w27(Wconv, bconv, scale):
    w = np.zeros((32, COUT), dtype=np.float32)
    for i, (c, dy, dx) in enumerate(TAPS):
        w[i] = scale * Wconv[:, c, dy, dx]
    w[27] = scale * bconv
    return w


def build_kernel(dbg=False):
    nc = bacc.Bacc("TRN2", target_bir_lowering=False, debug=False,
                   num_devices=NCORES)

    blob32 = nc.dram_tensor("blob32", [128, CB_COLS], F32, kind="ExternalInput")
    blob16 = nc.dram_tensor("blob16", [128, 128], F16, kind="ExternalInput")
    wblob = nc.dram_tensor("wblob", [32, 512], F32R, kind="ExternalInput")
    xa = nc.dram_tensor("xa", [28, NH], F32R, kind="ExternalInput")
    xb = nc.dram_tensor("xb", [28, NF], F32R, kind="ExternalInput")
    out = nc.dram_tensor("out", [1, 2], F32, kind="ExternalOutput")
    scr = nc.dram_tensor("scr", [128], I16, kind="Internal")
    rvd = nc.dram_tensor("rvd", [2048], F32, kind="Internal")
    fa_t = nc.dram_tensor("fa_t", [NH, COUT], F16, kind="Internal")

    if dbg:
        na_dbg = nc.dram_tensor("na_dbg", [128, 1], I32, kind="ExternalOutput")
        desc_dbg = nc.dram_tensor("desc_dbg", [128, 2, 128], F16, kind="ExternalOutput")
        nb_dbg = nc.dram_tensor("nb_dbg", [128, 1], I32, kind="ExternalOutput")
        drow_dbg = nc.dram_tensor("drow_dbg", [128, 2], F32, kind="ExternalOutput")

    with tile.TileContext(nc) as tc, ExitStack() as ctx:
        const = ctx.enter_context(tc.tile_pool(name="const", bufs=1))
        small = ctx.enter_context(tc.tile_pool(name="small", bufs=1))
        feat = ctx.enter_context(tc.tile_pool(name="feat", bufs=1))
        im_pool = ctx.enter_context(tc.tile_pool(name="im", bufs=4))
        fbt_pool = ctx.enter_context(tc.tile_pool(name="fbt", bufs=8))
        dram = ctx.enter_context(tc.tile_pool(name="dram", bufs=1, space="DRAM"))
        cq_pool = ctx.enter_context(tc.tile_pool(name="cq", bufs=2, space="PSUM"))
        cq2_pool = ctx.enter_context(tc.tile_pool(name="cq2", bufs=2, space="PSUM"))

        # ---- head: all input DMAs on the SP queue, deps-free, in use order ----
        cb32 = const.tile([128, CB_COLS], F32, tag="cb32")
        cb16 = const.tile([128, 128], F16, tag="cb16")
        cwb = const.tile([32, 512], F32R, tag="cwb")
        nc.sync.dma_start(cwb[:], wblob.ap())
        im_a = []
        for ha in range(2):
            t = im_pool.tile([32, NH // 2], F32R, tag="im")
            nc.sync.dma_start(
                t[0:28, :], AP(tensor=xa, offset=ha * (NH // 2),
                               ap=[[NH, 28], [1, NH // 2]]))
            im_a.append(t)
        im_b = [None] * 4

        def load_imb(hb):
            t = im_pool.tile([32, 4096], F32R, tag="im")
            nc.sync.dma_start(
                t[0:28, :], AP(tensor=xb, offset=hb * 4096,
                               ap=[[NF, 28], [1, 4096]]))
            im_b[hb] = t

        load_imb(0)
        load_imb(1)
        nc.sync.dma_start(cb32[:], blob32.ap())
        nc.sync.dma_start(cb16[:], blob16.ap())

        # const views
        w27a_v = cwb[0:28, 0:256]
        w27_v = cwb[0:28, 256:512]
        rowbl_v = cb32[:, CB_ROWBL : CB_ROWBL + 1].bitcast(I32)
        colb_v = cb32[:, CB_COLB : CB_COLB + 1].bitcast(I32)
        row64_v = cb32[:, CB_ROW64 : CB_ROW64 + 1].bitcast(I32)
        kiota_v = cb32[:, CB_KIOTA : CB_KIOTA + 1].bitcast(I32)
        rpat_v = cb32[:, CB_RPAT : CB_RPAT + 64].bitcast(I32)
        w1_v = cb32[:, CB_W1 : CB_W1 + 512].rearrange("p (a b c) -> p a b c", a=2, b=2)
        b1_v = cb32[:, CB_B1 : CB_B1 + 2]
        w2_v = cb32[:, CB_W2 : CB_W2 + 2]
        b2_v = cb32[0:1, CB_B2 : CB_B2 + 2]
        iota16_v = cb32[:, CB_IOTA16 : CB_IOTA16 + 16]
        nones_v = cb16[:, :]

        # ================= Phase 1: conv A (n-partitioned) ==========
        # 16 groups of 512 positions; psum [128n, 4u x 256c]; relu alternates
        # ACT/Pool into fanall; resp row-sums on DVE (fp32).
        fanall = feat.tile([128, 64, 256], F16)
        resp_nb = small.tile([128, 64], F32)
        for g in range(16):
            if g == 9:
                load_imb(2)   # reuses slot of im_a[0] (free after group 7)
            imt = im_a[g // 8]
            ps = (cq_pool if g % 2 == 0 else cq2_pool).tile([128, 1024], F32, tag="cps")
            for u in range(4):
                o = 512 * (g % 8) + 128 * u
                nc.tensor.matmul(ps[:, 256 * u : 256 * (u + 1)],
                                 imt[0:28, o : o + 128], w27a_v,
                                 start=True, stop=True)
            dst = fanall[:, 4 * g : 4 * (g + 1), :]
            nc.scalar.activation(dst.rearrange("p a b -> p (a b)"), ps[:], AF.Relu)
            nc.sync.dma_start(
                AP(tensor=fa_t, offset=4 * g * 128 * COUT,
                   ap=[[COUT, 128], [128 * COUT, 4], [1, COUT]]),
                dst)
            nc.vector.tensor_reduce(
                resp_nb[:, 4 * g : 4 * g + 4], dst,
                axis=mybir.AxisListType.X, op=ALU.add)

        load_imb(3)   # reuses slot of im_a[1] (free after group 15)

        # ---- selection: packed-bits tree (unchanged from v1) ----
        rpk = small.tile([128, 64], F32)
        nc.vector.tensor_single_scalar(rpk[:].bitcast(I32),
                                       resp_nb[:].bitcast(I32), -8,
                                       ALU.bitwise_and)
        nc.vector.tensor_tensor(rpk[:].bitcast(I32), rpk[:].bitcast(I32),
                                rpat_v, ALU.bitwise_or)
        rv = rpk[:].rearrange("p (gr r) -> p gr r", gr=8)
        t1 = small.tile([128, 8, 4], F32)
        t2 = small.tile([128, 8, 2], F32)
        rowwin = small.tile([128, 8], F32)
        nc.vector.tensor_tensor(t1[:], rv[:, :, 0:4], rv[:, :, 4:8], ALU.max)
        nc.vector.tensor_tensor(t2[:], t1[:, :, 0:2], t1[:, :, 2:4], ALU.max)
        nc.vector.tensor_tensor(rowwin[:].rearrange("p (gr o) -> p gr o", o=1),
                                t2[:, :, 0:1], t2[:, :, 1:2], ALU.max)
        nc.sync.dma_start(
            AP(tensor=rvd, offset=0, ap=[[8, 128], [1, 8]]), rowwin[:])
        bpk = small.tile([128, 8], F32)
        nc.sync.dma_start(
            bpk[:], AP(tensor=rvd, offset=0, ap=[[1, 8], [64, 16], [8, 8]]))

        vmax8 = small.tile([128, 8], F32)
        vidx8 = small.tile([128, 8], U32)
        tmpu = small.tile([128, 1], I32)
        tmpv = small.tile([128, 1], I32)
        rowa_l = small.tile([128, 1], I32)
        rowa_g = small.tile([128, 1], I32)
        cola_l = small.tile([128, 1], I32)
        na_l = small.tile([128, 1], I32)
        nc.vector.max(vmax8[:], bpk[:])
        nc.vector.max_index(vidx8[:], vmax8[:], bpk[:])
        nc.vector.tensor_single_scalar(tmpu[:], vmax8[:, 0:1].bitcast(I32), 7,
                                       ALU.bitwise_and)
        nc.vector.tensor_single_scalar(tmpu[:], tmpu[:], -1, ALU.mult)
        nc.vector.tensor_single_scalar(tmpu[:], tmpu[:], 7, ALU.add)
        nc.vector.tensor_copy(tmpv[:].bitcast(U32), vidx8[:, 0:1])
        nc.vector.tensor_tensor(rowa_l[:], rowbl_v, tmpu[:], ALU.add)
        nc.vector.tensor_tensor(cola_l[:], colb_v, tmpv[:], ALU.add)
        nc.vector.tensor_single_scalar(na_l[:], rowa_l[:], 7, ALU.logical_shift_left)
        nc.vector.tensor_tensor(na_l[:], na_l[:], cola_l[:], ALU.add)
        nc.vector.tensor_tensor(rowa_g[:], rowa_l[:], row64_v, ALU.add)
        if dbg:
            nc.scalar.dma_start(na_dbg.ap(), na_l[:])

        # idx wrap for the descriptor gather: scr roundtrip, single read.
        na_i16 = small.tile([128, 1], I16)
        idxw = small.tile([128, 8], I16)
        nc.vector.tensor_copy(na_i16[:], na_l[:])
        nc.sync.dma_start(AP(tensor=scr, offset=0, ap=[[1, 128]]), na_i16[:])
        for gq in range(8):
            eng = nc.sync if gq < 5 else nc.gpsimd
            eng.dma_start(
                idxw[16 * gq : 16 * (gq + 1), :],
                AP(tensor=scr, offset=0, ap=[[1, 16], [16, 8]]))

        desc_i = small.tile([128, 2, 128], F16)   # [c, mb, k] (pre-scaled x2)

        # ============== Phase 2: conv B + scoring, fine-interleaved ========
        tmax = small.tile([128, 16, 8], F32)
        tidx = small.tile([128, 16, 8], U32)
        fbs = {}
        NLEAD = 6

        def convb_group(g):
            imt = im_b[g // 4]
            sl = slice(1024 * (g % 4), 1024 * (g % 4) + 1024)
            fb = fbt_pool.tile([128, 2, 1024], F16, tag="fb")
            fb2 = fbt_pool.tile([128, 2, 1024], F16, tag="fb2")
            for mb in range(2):
                ps = cq_pool.tile([128, 1024], F32, tag="cps")
                for q in range(2):
                    nc.tensor.matmul(ps[:, 512 * q : 512 * (q + 1)],
                                     w27_v[:, 128 * mb : 128 * (mb + 1)],
                                     imt[0:28, sl][:, 512 * q : 512 * (q + 1)],
                                     start=True, stop=True)
                nc.scalar.activation(fb[:, mb, :], ps[:], AF.Relu)
            # squares: leads all-DVE (Pool must reach the desc gather first,
            # ACT is the conv-B relu pacer); steady state: Pool takes mb0,
            # ACT/DVE split mb1 2:1.
            if g < NLEAD:
                nc.vector.tensor_tensor(
                    fb2[:].rearrange("p a b -> p (a b)"),
                    fb[:].rearrange("p a b -> p (a b)"),
                    fb[:].rearrange("p a b -> p (a b)"), ALU.mult)
            else:
                nc.gpsimd.tensor_tensor(fb2[:, 0, :], fb[:, 0, :], fb[:, 0, :],
                                        ALU.mult)
                if g % 3 == 2:
                    nc.vector.tensor_tensor(fb2[:, 1, :], fb[:, 1, :],
                                            fb[:, 1, :], ALU.mult)
                else:
                    nc.scalar.square(fb2[:, 1, :], fb[:, 1, :])
            fbs[g] = (fb, fb2)

        def score_chunk(c):
            fb, fb2 = fbs.pop(c)
            sp = cq2_pool.tile([128, 1024], F32, tag="cps")
            for s in range(2):
                sl2 = slice(512 * s, 512 * (s + 1))
                po = sp[:, sl2]
                nc.tensor.matmul(po, desc_i[:, 0, :], fb[:, 0, sl2], start=True, stop=False)
                nc.tensor.matmul(po, desc_i[:, 1, :], fb[:, 1, sl2], start=False, stop=False)
                nc.tensor.matmul(po, nones_v, fb2[:, 0, sl2], start=False, stop=False)
                nc.tensor.matmul(po, nones_v, fb2[:, 1, sl2], start=False, stop=True)
            nc.vector.max(tmax[:, c, :], sp[:])
            nc.vector.max_index(tidx[:, c, :], tmax[:, c, :], sp[:])

        # Pool's first instruction is the descriptor gather; PE leads with
        # NLEAD conv groups, then interleaves 1 chunk : 1 group.
        nc.gpsimd.dma_gather(
            desc_i[:], fa_t.ap(), idxw[:],
            num_idxs=128, num_idxs_reg=128, elem_size=COUT, transpose=True)
        if dbg:
            nc.gpsimd.dma_start(desc_dbg.ap(), desc_i[:])
        for g in range(NLEAD):
            convb_group(g)
        for g in range(NLEAD, 16):
            score_chunk(g - NLEAD)
            convb_group(g)
        for c in range(16 - NLEAD, 16):
            score_chunk(c)

        # ---- global winner over the 16 chunk-top-1s ----
        gmx8 = small.tile([128, 8], F32)
        gix8 = small.tile([128, 8], U32)
        cstar = small.tile([128, 1], U32)
        cstarf = small.tile([128, 1], F32)
        mask16 = small.tile([128, 16], F32)
        locf = small.tile([128, 1], F32)
        locu = small.tile([128, 1], U32)
        nb = small.tile([128, 1], I32)
        tview = tmax[:].rearrange("p a b -> p (a b)")[:, ::8]
        iview = tidx[:].rearrange("p a b -> p (a b)")[:, ::8]
        nc.vector.max(gmx8[:], tview)
        nc.vector.max_index(gix8[:], gmx8[:], tview)
        nc.vector.tensor_copy(cstar[:], gix8[:, 0:1])
        nc.vector.tensor_copy(cstarf[:], cstar[:])
        nc.vector.tensor_scalar(mask16[:], iota16_v, cstarf[:], None, ALU.is_equal)
        nc.vector.tensor_tensor(mask16[:], mask16[:], iview, ALU.mult)
        nc.vector.tensor_reduce(locf[:], mask16[:], axis=mybir.AxisListType.X, op=ALU.add)
        nc.vector.tensor_copy(locu[:], locf[:])
        nc.vector.tensor_single_scalar(cstar[:], cstar[:], 10, ALU.logical_shift_left)
        nc.vector.tensor_tensor(nb[:].bitcast(U32), cstar[:], locu[:], ALU.add)
        if dbg:
            nc.scalar.dma_start(nb_dbg.ap(), nb[:])

        # ---- displacements ----
        rowb_t = small.tile([128, 1], I32)
        colb_t = small.tile([128, 1], I32)
        di_t = small.tile([128, 1], I32)
        d_f = small.tile([128, 2], F32)
        nc.vector.tensor_single_scalar(rowb_t[:], nb[:], 7, ALU.logical_shift_right)
        nc.vector.tensor_single_scalar(colb_t[:], nb[:], 127, ALU.bitwise_and)
        nc.vector.tensor_tensor(di_t[:], rowb_t[:], rowa_g[:], ALU.subtract)
        nc.vector.tensor_copy(d_f[:, 0:1], di_t[:])
        nc.vector.tensor_tensor(di_t[:], cola_l[:], colb_t[:], ALU.subtract)
        nc.vector.tensor_copy(d_f[:, 1:2], di_t[:])
        if dbg:
            nc.scalar.dma_start(drow_dbg.ap(), d_f[:])

        # ---- Exchange: AllGather displacement halves in the pair ----
        ex_in = dram.tile([128, 2], F32)
        ex_out = dram.tile([2, 128, 2], F32)
        nc.sync.dma_start(ex_in[:], d_f[:])
        nc.gpsimd.collective_compute(
            "AllGather", ALU.bypass,
            replica_groups=[[0, 1], [2, 3], [4, 5], [6, 7]],
            ins=[ex_in.opt()], outs=[ex_out.opt()])
        d_all = small.tile([128, 2, 2], F32)   # [k, half, rc]
        nc.sync.dma_start(d_all[:], ex_out[:].rearrange("r p c -> p r c"))

        # ---- MLPs ----
        out_sb = small.tile([1, 2], F32)
        hid = small.tile([128, 1], F32)
        b1s = small.tile([128, 2], F32)
        b2s = small.tile([1, 2], F32)
        nc.vector.tensor_copy(b1s[:], b1_v)
        nc.vector.tensor_copy(b2s[:], b2_v)
        for rc in range(2):
            hp = cq2_pool.tile([128, 1024], F32, tag="cps")
            for half in range(2):
                nc.tensor.matmul(hp[:, 0:1], w1_v[:, rc, half, :],
                                 d_all[:, half, rc : rc + 1],
                                 start=(half == 0), stop=(half == 1))
            nc.scalar.activation(hid[:], hp[:, 0:1], AF.Relu, bias=b1s[:, rc : rc + 1])
            op = cq2_pool.tile([128, 1024], F32, tag="cps")
            nc.tensor.matmul(op[:1, 0:1], hid[:], w2_v[:, rc : rc + 1],
                             start=True, stop=True)
            nc.scalar.activation(out_sb[:, rc : rc + 1], op[:1, 0:1], AF.Identity,
                                 bias=b2s[:, rc : rc + 1])
        nc.sync.dma_start(out.ap(), out_sb[:])

    nc.compile()
    return nc


_NC_CACHE = {}


def _get_nc(dbg=False):
    if dbg not in _NC_CACHE:
        _NC_CACHE[dbg] = build_kernel(dbg=dbg)
    return _NC_CACHE[dbg]


def _host_inputs(inputs):
    xA = np.asarray(inputs["xA"], np.float32)
    xB = np.asarray(inputs["xB"], np.float32)
    Wc = np.asarray(inputs["Wconv"], dtype=np.float32)
    bc = np.asarray(inputs["bconv"], dtype=np.float32)

    blob32 = np.zeros((128, CB_COLS), dtype=np.float32)
    bi = blob32.view(np.int32)
    wblob = np.zeros((32, 512), dtype=np.float32)
    wblob[:, 0:256] = _prep_w27(Wc, bc, 4.0)
    wblob[:, 256:512] = _prep_w27(Wc, bc, 2.0)
    p = np.arange(128)
    bi[:, CB_ROWBL] = (8 * (p // 16)).astype(np.int32)
    bi[:, CB_COLB] = (8 * (p % 16)).astype(np.int32)
    bi[:, CB_KIOTA] = p.astype(np.int32)
    bi[:, CB_RPAT : CB_RPAT + 64] = np.broadcast_to(
        7 - (np.arange(64, dtype=np.int32) % 8), (128, 64))
    w1 = np.stack([
        np.asarray(inputs["W1r"], np.float32).reshape(2, 128, 128),
        np.asarray(inputs["W1c"], np.float32).reshape(2, 128, 128),
    ])  # [rc, half, p, j]
    blob32[:, CB_W1 : CB_W1 + 512] = w1.transpose(2, 0, 1, 3).reshape(128, 512)
    blob32[:, CB_B1 : CB_B1 + 2] = np.stack(
        [np.asarray(inputs["b1r"], np.float32), np.asarray(inputs["b1c"], np.float32)], 1)
    blob32[:, CB_W2 : CB_W2 + 2] = np.concatenate(
        [np.asarray(inputs["W2r"], np.float32), np.asarray(inputs["W2c"], np.float32)], 1)
    blob32[0, CB_B2 : CB_B2 + 2] = [float(np.asarray(inputs["b2r"])[0]),
                                    float(np.asarray(inputs["b2c"])[0])]
    blob32[:, CB_IOTA16 : CB_IOTA16 + 16] = np.arange(16, dtype=np.float32)[None, :]
    blob16 = -np.ones((128, 128), dtype=np.float16)

    in_maps = []
    for c in range(NCORES):
        b, par = c // 2, c % 2
        m = dict(blob32=blob32, blob16=blob16, wblob=wblob)
        m["xa"] = _prep_planes(xA[b], 64 * par, 64)
        m["xb"] = _prep_planes(xB[b], 0, 128)
        bl = blob32.copy()
        bl.view(np.int32)[:, CB_ROW64] = 64 * par
        m["blob32"] = bl
        in_maps.append(m)
    return in_maps


def kernel(**inputs):
    nc = _get_nc(dbg=False)
    in_maps = _host_inputs(inputs)
    res = bass_utils.run_bass_kernel_spmd(nc, in_maps, core_ids=list(range(NCORES)))
    return np.concatenate([res.results[2 * b]["out"] for b in range(B)], axis=0)


def kernel_dbg(**inputs):
    nc = _get_nc(dbg=True)
    in_maps = _host_inputs(inputs)
    res = bass_utils.run_bass_kernel_spmd(nc, in_maps, core_ids=list(range(NCORES)))
    out = np.concatenate([res.results[2 * b]["out"] for b in range(B)], axis=0)
    return out, res.results
